# revision 39
# baseline (speedup 1.0000x reference)
"""Multi-head attention on 8 TRN2 NeuronCores (Bass/Tile).

Sharding: core c handles batch b = c//2 and query-half h = c%2 (1024 query
tokens), all 16 heads. K/V projections are per-batch and duplicated across
the two cores sharing a batch; no cross-core communication.

Design notes (v3):
- All matmul operands are bf16 (1 PE cycle/row at any output width, vs
  fp32r's 4x penalty below 256), halving DMA traffic as well. PSUM stays
  fp32.
- Keys are compacted on the host using the 0/1 key mask (masked keys
  dropped, padded to a multiple of 128, Kpad). Pad positions are killed by
  a -1e9 per-partition bias folded into the Exp activation.
- The V-projection bias is folded into the output-projection bias on the
  host (bo' = bo + Wo @ bv), since softmax weights sum to 1.
- Energy is computed transposed ([key, query] tiles). AV is computed as
  out[q-tile, 65] accumulating over key tiles: each head's V tile carries
  an extra ones column, so column 64 of the PSUM output is the softmax
  denominator, a per-partition scalar. Normalization is then a [128,k]
  reciprocal plus per-head tensor_scalar multiplies - no broadcast matmul.
  Normalized O ([q, e] layout) returns to [e, q] tiles for the output
  projection via XBAR DMA transposes on the otherwise-idle SP queue.
- Attention runs in (head-pair, query-half) units, fused at key-tile
  granularity: each kt step carries the energy matmuls + exp of (u, kt)
  and the AV matmuls of unit u-1, all at high scheduler priority, while
  the Q/K/V projections are emitted as ~1.7us single-bank PSUM chunks
  that fill PE gaps whenever the Act engine's exp stream (the secondary
  bottleneck, ~150us) falls behind.
- All Q projections precede Kp(0) so the filler-PSUM rotation never
  stalls behind the late wk DMA; emission order matters beyond
  priorities because tile dependency tracking is program-order
  directional (a read emitted before its writer races it).
"""

import sys

sys.path.insert(0, "/opt/trn_rl_repo")

from contextlib import ExitStack

import ml_dtypes
import numpy as np

import concourse.bass as bass  # noqa: F401
import concourse.tile as tile
from concourse import bacc, mybir
from concourse.bass_utils import run_bass_kernel_spmd

E = 1024          # embed dim
HEADS = 16
HD = 64           # head dim
B = 4
S = 2048
NCORES = 8
Q = (B * S) // NCORES  # query tokens per core
EC = E // 128     # embed chunks of 128
F32 = mybir.dt.float32
BF16 = mybir.dt.bfloat16
BF16NP = ml_dtypes.bfloat16


def build_program(Kpad):
    """Build the per-core Bass program (identical on all 8 cores)."""
    KTn = Kpad // 128
    nc = bacc.Bacc("TRN2", target_bir_lowering=False, debug=False,
                   num_devices=NCORES, dynamic_dma_scratch_size=2048)

    qT = nc.dram_tensor("qT", [E, Q], BF16, kind="ExternalInput").ap()
    kT = nc.dram_tensor("kT", [E, Kpad], BF16, kind="ExternalInput").ap()
    vT = nc.dram_tensor("vT", [E, Kpad], BF16, kind="ExternalInput").ap()
    wqT = nc.dram_tensor("wqT", [E, E], BF16, kind="ExternalInput").ap()
    wkT = nc.dram_tensor("wkT", [E, E], BF16, kind="ExternalInput").ap()
    wvT = nc.dram_tensor("wvT", [E, E], BF16, kind="ExternalInput").ap()
    woT = nc.dram_tensor("woT", [E, E], BF16, kind="ExternalInput").ap()
    bq2 = nc.dram_tensor("bq2", [128, EC], F32, kind="ExternalInput").ap()
    bk2 = nc.dram_tensor("bk2", [128, EC], F32, kind="ExternalInput").ap()
    bo2 = nc.dram_tensor("bo2", [128, EC], F32, kind="ExternalInput").ap()
    mb = nc.dram_tensor("mb", [128, KTn], F32, kind="ExternalInput").ap()
    yT = nc.dram_tensor("yT", [E, Q], F32, kind="ExternalOutput").ap()

    # K-projection free-dim chunks (each must stay inside a 512-float bank)
    k_tail = Kpad - 1024 if Kpad > 1024 else 0

    with tile.TileContext(nc) as tc, ExitStack() as ctx:
        sml = ctx.enter_context(tc.tile_pool(name="sml", bufs=1))
        big = ctx.enter_context(tc.tile_pool(name="big", bufs=1))

        # ---- PSUM pools: psE 2x[128,1024]=4 banks, psA 2x1=2, psT 2x1=2
        psE = ctx.enter_context(tc.tile_pool(name="psE", bufs=2, space="PSUM"))
        psA = ctx.enter_context(tc.tile_pool(name="psA", bufs=1, space="PSUM"))
        psF = ctx.enter_context(tc.tile_pool(name="psF", bufs=2, space="PSUM"))

        inp = ctx.enter_context(tc.tile_pool(name="inp", bufs=1))
        pp = ctx.enter_context(tc.tile_pool(name="pp", bufs=1))
        nrm = ctx.enter_context(tc.tile_pool(name="nrm", bufs=4))
        # vt/wv free mid-attention; inpV sits atop the pool stack so it can
        inpV_ctx = ExitStack()
        inpV = inpV_ctx.enter_context(tc.tile_pool(name="inpV", bufs=1))

        # ---- big input DMAs (SP queue), in consumption order -----------
        # qT/wq are DMA'd per 128-row chunk so the first matmuls start
        # early; the small constants go out after the first two chunk
        # pairs (they are only needed once the first PSUM drains).
        qt_t, wq_t = [], []
        qv = qT[:].rearrange("(c p) q -> p c q", p=128)
        wv = wqT[:].rearrange("(c p) e -> p c e", p=128)
        bq_s = sml.tile([128, EC], F32, name="bq_s")
        bk_s = sml.tile([128, EC], F32, name="bk_s")
        bo_s = sml.tile([128, EC], F32, name="bo_s")
        mb_s = sml.tile([128, KTn], F32, name="mb_s")
        for h in (0, 1):
            t = inp.tile([128, 4, Q], BF16, name=f"qt{h}")
            w = inp.tile([128, 4, E], BF16, name=f"wq{h}")
            for kk in range(4):
                nc.sync.dma_start(t[:, kk, :], qv[:, 4 * h + kk, :])
                nc.sync.dma_start(w[:, kk, :], wv[:, 4 * h + kk, :])
                if h == 0 and kk == 1:
                    nc.sync.dma_start(bq_s[:], bq2[:])
                    nc.sync.dma_start(bk_s[:], bk2[:])
                    nc.sync.dma_start(bo_s[:], bo2[:])
                    nc.sync.dma_start(mb_s[:], mb[:])
            qt_t.append(t)
            wq_t.append(w)
        kt_t = inp.tile([128, EC, Kpad], BF16, name="kt")
        nc.sync.dma_start(kt_t[:], kT[:].rearrange("(c p) k -> p c k", p=128))
        wk_t = inp.tile([128, EC, E], BF16, name="wk")
        nc.sync.dma_start(wk_t[:], wkT[:].rearrange("(c p) e -> p c e", p=128))
        vt_t = inpV.tile([128, EC, Kpad], BF16, name="vt")
        nc.sync.dma_start(vt_t[:], vT[:].rearrange("(c p) k -> p c k", p=128))
        wv_t = inpV.tile([128, EC, E], BF16, name="wv")
        nc.sync.dma_start(wv_t[:], wvT[:].rearrange("(c p) e -> p c e", p=128))

        # ---- persistent SBUF tensors -----------------------------------
        QTs = [big.tile([128, Q], BF16, name=f"QT{m}") for m in range(EC)]
        KTs = [big.tile([128, Kpad], BF16, name=f"KT{m}") for m in range(EC)]
        VVs = [big.tile([128, HEADS * 65], BF16, name=f"VV{t}")
               for t in range(KTn)]
        OTs = [big.tile([128, Q], BF16, name=f"OT{m}") for m in range(EC)]
        # ones columns of the V tiles, written once before anything reads VV
        for t in range(KTn):
            vv3 = VVs[t][:].rearrange("p (h e) -> p h e", e=65)
            nc.vector.memset(vv3[:, :, 64:65], 1.0)

        # ---- projection emitters ---------------------------------------
        # All projections run as 8-matmul chunks into a single-bank psF
        # tile that is consumed (DVE) right away, so low-priority filler
        # never blocks the high-priority energy PSUM rotation.
        def q_chunk(m, n0, k0=0, nk=EC, first=True):
            ps = psF.tile([128, 512], F32, tag="f", name=f"psq{m}_{n0}_{k0}")
            for kk in range(k0, k0 + nk):
                nc.tensor.matmul(
                    ps[:], wq_t[kk // 4][:, kk % 4, m * 128:(m + 1) * 128],
                    qt_t[kk // 4][:, kk % 4, n0:n0 + 512],
                    start=(kk == k0), stop=(kk == k0 + nk - 1))
            if first:
                nc.vector.tensor_scalar_add(
                    QTs[m][:, n0:n0 + 512], ps[:], bq_s[:, m:m + 1])
            else:
                with nc.allow_low_precision(reason="bf16 proj accum"):
                    nc.vector.tensor_add(
                        QTs[m][:, n0:n0 + 512], QTs[m][:, n0:n0 + 512], ps[:])

        def k_chunk(m, n0, nn=512):
            ps = psF.tile([128, 512], F32, tag="f", name=f"psk{m}_{n0}")
            for kk in range(EC):
                nc.tensor.matmul(
                    ps[:, 0:nn], wk_t[:, kk, m * 128:(m + 1) * 128],
                    kt_t[:, kk, n0:n0 + nn],
                    start=(kk == 0), stop=(kk == EC - 1))
            nc.vector.tensor_scalar_add(
                KTs[m][:, n0:n0 + nn], ps[:, 0:nn], bk_s[:, m:m + 1])

        def v_chunk(t, half):
            """V-proj chunk: heads half*8..half*8+8 of key tile t."""
            n0 = half * 512
            ps = psF.tile([128, 512], F32, tag="f", name=f"psv{t}_{half}")
            for kk in range(EC):
                nc.tensor.matmul(
                    ps[:], vt_t[:, kk, t * 128:(t + 1) * 128],
                    wv_t[:, kk, n0:n0 + 512],
                    start=(kk == 0), stop=(kk == EC - 1))
            vv3 = VVs[t][:].rearrange("p (h e) -> p h e", e=65)
            ps3 = ps[:].rearrange("p (h d) -> p h d", d=64)
            nc.vector.tensor_copy(vv3[:, half * 8:half * 8 + 8, 0:64], ps3[:])

        def proj_chunks(m):
            out = [lambda n0=n0: k_chunk(m, n0) for n0 in (0, 512)]
            if k_tail:
                out.append(lambda: k_chunk(m, 1024, k_tail))
            return out

        # ---- attention: units of (head pair j, query half qh) ----------
        # Emission is fused at kt granularity: each kt step of unit u carries
        # the energy matmuls + exp of (u, kt) AND the AV matmuls of unit u-1
        # at kt, so the in-order PE queue always has AV filler between
        # energy steps while the Act engine's exp stream catches up.

        def emit_av_kt(st, kt):
            """AV matmuls at key-tile kt for the unit described by `st`."""
            j, qh, ptiles, avs = st["j"], st["qh"], st["pt"], st["avs"]
            for pair in (0, 1):
                av = avs[pair]
                for qi in (0, 1):
                    for hh in (0, 1):
                        idx = kt * 4 + qi * 2 + hh
                        q0 = hh * 512 + (pair * 2 + qi) * 128
                        nc.tensor.matmul(
                            av[:, (qi * 2 + hh) * 65:(qi * 2 + hh + 1) * 65],
                            ptiles[kt][:, q0:q0 + 128],
                            VVs[kt][:, (2 * j + hh) * 65:(2 * j + hh + 1) * 65],
                            start=(idx == 0), stop=(idx == KTn * 4 - 1),
                            skip_group_check=True)

        def finalize_av(st):
            """Normalize + transpose the finished AV unit `st`."""
            j, qh, avs, oj = st["j"], st["qh"], st["avs"], st["oj"]
            for pair in (0, 1):
                av = avs[pair]
                av3 = av[:].rearrange("p (x c) -> p x c", c=65)
                rc = nrm.tile([128, 4], F32, tag="rc",
                              name=f"rc{j}_{qh}_{pair}")
                nc.vector.reciprocal(
                    rc[:].rearrange("p (a b) -> p a b", b=1), av3[:, :, 64:65])
                for qi in (0, 1):
                    ql = pair * 2 + qi
                    qc = qh * 4 + ql
                    for hh in (0, 1):
                        i = qi * 2 + hh
                        nc.vector.tensor_scalar_mul(
                            oj[:, ql, hh * 64:hh * 64 + 64],
                            av[:, i * 65:i * 65 + 64], rc[:, i:i + 1])
                    nc.sync.dma_start_transpose(
                        OTs[j][:, qc * 128:(qc + 1) * 128], oj[:, ql, :])

        HIPRI = 1 << 20
        for k in range(EC):
            for n0 in (0, 512):
                q_chunk(0, n0, k, 1, k == 0)
        # all remaining Q projections precede Kp(0) so the psF buffer
        # rotation never stalls behind the (late) wk DMA
        for m in range(1, EC):
            q_chunk(m, 0)
            q_chunk(m, 512)
        k_chunk(0, 0)
        k_chunk(0, 512)
        if k_tail:
            k_chunk(0, 1024, k_tail)
        units = [(j, qh) for j in range(EC) for qh in (0, 1)]
        wo_t = [None]
        pa_t = []
        prev = None
        for j, qh in units:
            ptiles = []
            chunks = (proj_chunks(j + 1)
                      if (qh == 0 and j < EC - 1) else [])
            if (j, qh) == (0, 1):
                # V projection must be emitted before any AV(0,0) matmul:
                # dependency tracking is program-order directional, so a
                # read emitted before its writer would race it.
                for t in range(KTn):
                    v_chunk(t, 0)
                    v_chunk(t, 1)
                inpV_ctx.close()
                wo_pool = ctx.enter_context(tc.tile_pool(name="wop",
                                                         bufs=1))
                wo_t[0] = wo_pool.tile([128, EC, E], BF16, name="wo")
                nc.sync.dma_start(
                    wo_t[0][:],
                    woT[:].rearrange("(c p) e -> p c e", p=128))
                for m in range(EC):
                    pa_t.append(wo_pool.tile([128, Q], BF16, name=f"pa{m}"))
            for kt in range(KTn):
                with tc.high_priority(offset=HIPRI):
                    pe = psE.tile([128, 1024], F32, tag="e",
                                  name=f"pe{j}_{qh}_{kt}")
                    for hh in (0, 1):
                        off = hh * 64
                        nc.tensor.matmul(
                            pe[:, hh * 512:hh * 512 + 512],
                            KTs[j][off:off + 64, kt * 128:(kt + 1) * 128],
                            QTs[j][off:off + 64, qh * 512:qh * 512 + 512])
                    pt = pp.tile([128, 1024], BF16, tag=f"P{qh}_{kt}",
                                 name=f"pt{j}_{qh}_{kt}")
                    nc.scalar.activation(
                        pt[:], pe[:], mybir.ActivationFunctionType.Exp,
                        bias=mb_s[:, kt:kt + 1], scale=0.125)
                    ptiles.append(pt)
                    if prev is not None:
                        if kt == 0:
                            prev["avs"] = [
                                psA.tile([128, 260], F32, tag=f"a{pr}",
                                         name=f"av{prev['j']}_{prev['qh']}_{pr}")
                                for pr in (0, 1)]
                        emit_av_kt(prev, kt)
                if kt % 2 == 1 and chunks:
                    chunks.pop(0)()
            while chunks:
                chunks.pop(0)()
            if prev is not None:
                with tc.high_priority(offset=HIPRI):
                    finalize_av(prev)
            if (j, qh) == (6, 1):
                # first half of the output projection (pairs 0-3 done, all
                # K chunks emitted): fills pair-7's Act-paced PE gaps and
                # halves the post-attention serial tail
                for m in range(EC):
                    for n0 in (0, 512):
                        ps = psF.tile([128, 512], F32, tag="f",
                                      name=f"psya{m}_{n0}")
                        for k in range(4):
                            nc.tensor.matmul(
                                ps[:], wo_t[0][:, k, m * 128:(m + 1) * 128],
                                OTs[k][:, n0:n0 + 512],
                                start=(k == 0), stop=(k == 3))
                        nc.vector.tensor_copy(pa_t[m][:, n0:n0 + 512], ps[:])
            prev = dict(j=j, qh=qh, pt=ptiles, avs=None,
                        oj=nrm.tile([128, 4, 128], BF16, tag="oj",
                                    name=f"oj{j}_{qh}"))
        with tc.high_priority(offset=HIPRI):
            prev["avs"] = [psA.tile([128, 260], F32, tag=f"a{pr}",
                                    name=f"av_last_{pr}") for pr in (0, 1)]
            for kt in range(KTn):
                emit_av_kt(prev, kt)
            finalize_av(prev)

        out_pool = ctx.enter_context(tc.tile_pool(name="outp", bufs=2))

        # ---- output projection, second half + combine ------------------
        for m in range(EC):
            for n0 in (0, 512):
                ps = psF.tile([128, 512], F32, tag="f", name=f"psyb{m}_{n0}")
                for k in range(4, EC):
                    nc.tensor.matmul(
                        ps[:], wo_t[0][:, k, m * 128:(m + 1) * 128],
                        OTs[k][:, n0:n0 + 512],
                        start=(k == 4), stop=(k == EC - 1))
                yt = out_pool.tile([128, 512], F32, tag="yt",
                                   name=f"yt{m}_{n0}")
                nc.vector.tensor_scalar_add(yt[:], ps[:], bo_s[:, m:m + 1])
                with nc.allow_low_precision(reason="bf16 outproj partial"):
                    nc.vector.tensor_add(yt[:], yt[:],
                                         pa_t[m][:, n0:n0 + 512])
                nc.sync.dma_start(yT[m * 128:(m + 1) * 128, n0:n0 + 512],
                                  yt[:])

    nc.compile()
    return nc


_PROG_CACHE = {}


def _get_program(Kpad):
    if Kpad not in _PROG_CACHE:
        _PROG_CACHE[Kpad] = build_program(Kpad)
    return _PROG_CACHE[Kpad]


def prepare_inputs(query, keys, values, mask, Wq, bq, Wk, bk, Wv, bv, Wo, bo):
    """Host-side sharding/layout prep. Returns (Kpad, in_maps)."""
    f32 = np.float32
    query = np.asarray(query, f32)
    keys = np.asarray(keys, f32)
    values = np.asarray(values, f32)
    mask = np.asarray(mask)

    idxs = [np.nonzero(mask[b] != 0)[0] for b in range(B)]
    nmax = max(len(i) for i in idxs)
    Kpad = max(256, ((max(nmax, 1) + 127) // 128) * 128)
    KTn = Kpad // 128

    kTb = np.zeros((B, E, Kpad), BF16NP)
    vTb = np.zeros((B, E, Kpad), BF16NP)
    mbb = np.full((B, Kpad), -1e9, f32)
    for b in range(B):
        n = len(idxs[b])
        kTb[b, :, :n] = keys[b][idxs[b]].T.astype(BF16NP)
        vTb[b, :, :n] = values[b][idxs[b]].T.astype(BF16NP)
        mbb[b, :n] = 0.0
    mb2 = np.ascontiguousarray(mbb.reshape(B, KTn, 128).transpose(0, 2, 1))

    WqT = np.ascontiguousarray(np.asarray(Wq, f32).T.astype(BF16NP))
    WkT = np.ascontiguousarray(np.asarray(Wk, f32).T.astype(BF16NP))
    WvT = np.ascontiguousarray(np.asarray(Wv, f32).T.astype(BF16NP))
    WoT = np.ascontiguousarray(np.asarray(Wo, f32).T.astype(BF16NP))
    bq2 = np.ascontiguousarray(np.asarray(bq, f32).reshape(EC, 128).T)
    bk2 = np.ascontiguousarray(np.asarray(bk, f32).reshape(EC, 128).T)
    # fold V bias through the output projection: y += (Wo @ bv + bo)
    bo_f = np.asarray(bo, f32) + np.asarray(Wo, f32) @ np.asarray(bv, f32)
    bo2 = np.ascontiguousarray(bo_f.reshape(EC, 128).T)

    in_maps = []
    for c in range(NCORES):
        b, h = c // 2, c % 2
        in_maps.append(dict(
            qT=np.ascontiguousarray(
                query[b, h * Q:(h + 1) * Q, :].T.astype(BF16NP)),
            kT=kTb[b], vT=vTb[b], mb=mb2[b],
            wqT=WqT, wkT=WkT, wvT=WvT, woT=WoT,
            bq2=bq2, bk2=bk2, bo2=bo2,
        ))
    return Kpad, in_maps


def kernel(query, keys, values, mask, Wq, bq, Wk, bk, Wv, bv, Wo, bo):
    Kpad, in_maps = prepare_inputs(query, keys, values, mask,
                                   Wq, bq, Wk, bk, Wv, bv, Wo, bo)
    nc = _get_program(Kpad)
    res = run_bass_kernel_spmd(nc, in_maps, list(range(NCORES)))
    out = np.empty((B, S, E), np.float32)
    for c in range(NCORES):
        b, h = c // 2, c % 2
        out[b, h * Q:(h + 1) * Q, :] = res.results[c]["yT"].T
    return out


# revision 40
# speedup vs baseline: 1.0489x; 1.0489x over previous
"""Multi-head attention on 8 TRN2 NeuronCores (Bass/Tile).

Sharding: core c handles batch b = c//2 and query-half h = c%2 (1024 query
tokens), all 16 heads. K/V projections are per-batch and duplicated across
the two cores sharing a batch; no cross-core communication.

Design notes (v3):
- All matmul operands are bf16 (1 PE cycle/row at any output width, vs
  fp32r's 4x penalty below 256), halving DMA traffic as well. PSUM stays
  fp32.
- Keys are compacted on the host using the 0/1 key mask (masked keys
  dropped, padded to a multiple of 128, Kpad). Pad positions are killed by
  a -1e9 per-partition bias folded into the Exp activation.
- The V-projection bias is folded into the output-projection bias on the
  host (bo' = bo + Wo @ bv), since softmax weights sum to 1.
- Energy is computed transposed ([key, query] tiles). AV is computed as
  out[q-tile, 65] accumulating over key tiles: each head's V tile carries
  an extra ones column, so column 64 of the PSUM output is the softmax
  denominator, a per-partition scalar. Normalization is then a [128,k]
  reciprocal plus per-head tensor_scalar multiplies - no broadcast matmul.
  Normalized O ([q, e] layout) returns to [e, q] tiles for the output
  projection via XBAR DMA transposes on the otherwise-idle SP queue.
- Attention runs in (head-pair, query-half) units, fused at key-tile
  granularity: each kt step carries the energy matmuls + exp of (u, kt)
  and the AV matmuls of unit u-1, all at high scheduler priority, while
  the Q/K/V projections are emitted as ~1.7us single-bank PSUM chunks
  that fill PE gaps whenever the Act engine's exp stream (the secondary
  bottleneck, ~150us) falls behind.
- All Q projections precede Kp(0) so the filler-PSUM rotation never
  stalls behind the late wk DMA; emission order matters beyond
  priorities because tile dependency tracking is program-order
  directional (a read emitted before its writer races it).
"""

import sys

sys.path.insert(0, "/opt/trn_rl_repo")

from contextlib import ExitStack

import ml_dtypes
import numpy as np

import concourse.bass as bass  # noqa: F401
import concourse.tile as tile
from concourse import bacc, mybir
from concourse.bass_utils import run_bass_kernel_spmd

E = 1024          # embed dim
HEADS = 16
HD = 64           # head dim
B = 4
S = 2048
NCORES = 8
Q = (B * S) // NCORES  # query tokens per core
EC = E // 128     # embed chunks of 128
F32 = mybir.dt.float32
BF16 = mybir.dt.bfloat16
BF16NP = ml_dtypes.bfloat16


def build_program(Kpad):
    """Build the per-core Bass program (identical on all 8 cores)."""
    KTn = Kpad // 128
    nc = bacc.Bacc("TRN2", target_bir_lowering=False, debug=False,
                   num_devices=NCORES, dynamic_dma_scratch_size=2048)

    qT = nc.dram_tensor("qT", [E, Q], BF16, kind="ExternalInput").ap()
    kT = nc.dram_tensor("kT", [E, Kpad], BF16, kind="ExternalInput").ap()
    vT = nc.dram_tensor("vT", [E, Kpad], BF16, kind="ExternalInput").ap()
    wqT = nc.dram_tensor("wqT", [E, E], BF16, kind="ExternalInput").ap()
    wkT = nc.dram_tensor("wkT", [E, E], BF16, kind="ExternalInput").ap()
    wvT = nc.dram_tensor("wvT", [E, E], BF16, kind="ExternalInput").ap()
    woT = nc.dram_tensor("woT", [E, E], BF16, kind="ExternalInput").ap()
    bq2 = nc.dram_tensor("bq2", [128, EC], F32, kind="ExternalInput").ap()
    bk2 = nc.dram_tensor("bk2", [128, EC], F32, kind="ExternalInput").ap()
    bo2 = nc.dram_tensor("bo2", [128, EC], F32, kind="ExternalInput").ap()
    mb = nc.dram_tensor("mb", [128, KTn], F32, kind="ExternalInput").ap()
    yT = nc.dram_tensor("yT", [E, Q], F32, kind="ExternalOutput").ap()

    # K-projection free-dim chunks (each must stay inside a 512-float bank)
    k_tail = Kpad - 1024 if Kpad > 1024 else 0

    with tile.TileContext(nc) as tc, ExitStack() as ctx:
        sml = ctx.enter_context(tc.tile_pool(name="sml", bufs=1))
        big = ctx.enter_context(tc.tile_pool(name="big", bufs=1))

        # ---- PSUM pools: psE 2x[128,1024]=4 banks, psA 2x1=2, psT 2x1=2
        psE = ctx.enter_context(tc.tile_pool(name="psE", bufs=2, space="PSUM"))
        psA = ctx.enter_context(tc.tile_pool(name="psA", bufs=1, space="PSUM"))
        psF = ctx.enter_context(tc.tile_pool(name="psF", bufs=2, space="PSUM"))

        inp = ctx.enter_context(tc.tile_pool(name="inp", bufs=1))
        pp = ctx.enter_context(tc.tile_pool(name="pp", bufs=1))
        nrm = ctx.enter_context(tc.tile_pool(name="nrm", bufs=4))
        # vt/wv free mid-attention; inpV sits atop the pool stack so it can
        inpV_ctx = ExitStack()
        inpV = inpV_ctx.enter_context(tc.tile_pool(name="inpV", bufs=1))

        # ---- big input DMAs (SP queue), in consumption order -----------
        # qT/wq are DMA'd per 128-row chunk so the first matmuls start
        # early; the small constants go out after the first two chunk
        # pairs (they are only needed once the first PSUM drains).
        qt_t, wq_t = [], []
        qv = qT[:].rearrange("(c p) q -> p c q", p=128)
        wv = wqT[:].rearrange("(c p) e -> p c e", p=128)
        bq_s = sml.tile([128, EC], F32, name="bq_s")
        bk_s = sml.tile([128, EC], F32, name="bk_s")
        bo_s = sml.tile([128, EC], F32, name="bo_s")
        mb_s = sml.tile([128, KTn], F32, name="mb_s")
        for h in (0, 1):
            t = inp.tile([128, 4, Q], BF16, name=f"qt{h}")
            w = inp.tile([128, 4, E], BF16, name=f"wq{h}")
            for kk in range(4):
                nc.sync.dma_start(t[:, kk, :], qv[:, 4 * h + kk, :])
                nc.sync.dma_start(w[:, kk, :], wv[:, 4 * h + kk, :])
                if h == 0 and kk == 1:
                    nc.sync.dma_start(bq_s[:], bq2[:])
                    nc.sync.dma_start(bk_s[:], bk2[:])
                    nc.sync.dma_start(bo_s[:], bo2[:])
                    nc.sync.dma_start(mb_s[:], mb[:])
            qt_t.append(t)
            wq_t.append(w)
        kt_t = inp.tile([128, EC, Kpad], BF16, name="kt")
        nc.sync.dma_start(kt_t[:], kT[:].rearrange("(c p) k -> p c k", p=128))
        wk_t = inp.tile([128, EC, E], BF16, name="wk")
        nc.sync.dma_start(wk_t[:], wkT[:].rearrange("(c p) e -> p c e", p=128))
        vt_t = inpV.tile([128, EC, Kpad], BF16, name="vt")
        nc.sync.dma_start(vt_t[:], vT[:].rearrange("(c p) k -> p c k", p=128))
        wv_t = inpV.tile([128, EC, E], BF16, name="wv")
        nc.sync.dma_start(wv_t[:], wvT[:].rearrange("(c p) e -> p c e", p=128))

        # ---- persistent SBUF tensors -----------------------------------
        QTs = [big.tile([128, Q], BF16, name=f"QT{m}") for m in range(EC)]
        KTs = [big.tile([128, Kpad], BF16, name=f"KT{m}") for m in range(EC)]
        VVs = [big.tile([128, HEADS * 65], BF16, name=f"VV{t}")
               for t in range(KTn)]
        OTs = [big.tile([128, Q], BF16, name=f"OT{m}") for m in range(EC)]
        # ones columns of the V tiles, written once before anything reads VV
        for t in range(KTn):
            vv3 = VVs[t][:].rearrange("p (h e) -> p h e", e=65)
            nc.vector.memset(vv3[:, :, 64:65], 1.0)

        # ---- projection emitters ---------------------------------------
        # All projections run as 8-matmul chunks into a single-bank psF
        # tile that is consumed (DVE) right away, so low-priority filler
        # never blocks the high-priority energy PSUM rotation.
        def q_chunk(m, n0, k0=0, nk=EC, first=True):
            ps = psF.tile([128, 512], F32, tag="f", name=f"psq{m}_{n0}_{k0}")
            for kk in range(k0, k0 + nk):
                nc.tensor.matmul(
                    ps[:], wq_t[kk // 4][:, kk % 4, m * 128:(m + 1) * 128],
                    qt_t[kk // 4][:, kk % 4, n0:n0 + 512],
                    start=(kk == k0), stop=(kk == k0 + nk - 1))
            if first:
                nc.vector.tensor_scalar_add(
                    QTs[m][:, n0:n0 + 512], ps[:], bq_s[:, m:m + 1])
            else:
                with nc.allow_low_precision(reason="bf16 proj accum"):
                    nc.vector.tensor_add(
                        QTs[m][:, n0:n0 + 512], QTs[m][:, n0:n0 + 512], ps[:])

        def k_chunk(m, n0, nn=512):
            ps = psF.tile([128, 512], F32, tag="f", name=f"psk{m}_{n0}")
            for kk in range(EC):
                nc.tensor.matmul(
                    ps[:, 0:nn], wk_t[:, kk, m * 128:(m + 1) * 128],
                    kt_t[:, kk, n0:n0 + nn],
                    start=(kk == 0), stop=(kk == EC - 1))
            nc.vector.tensor_scalar_add(
                KTs[m][:, n0:n0 + nn], ps[:, 0:nn], bk_s[:, m:m + 1])

        def v_chunk(t, half):
            """V-proj chunk: heads half*8..half*8+8 of key tile t."""
            n0 = half * 512
            ps = psF.tile([128, 512], F32, tag="f", name=f"psv{t}_{half}")
            for kk in range(EC):
                nc.tensor.matmul(
                    ps[:], vt_t[:, kk, t * 128:(t + 1) * 128],
                    wv_t[:, kk, n0:n0 + 512],
                    start=(kk == 0), stop=(kk == EC - 1))
            vv3 = VVs[t][:].rearrange("p (h e) -> p h e", e=65)
            ps3 = ps[:].rearrange("p (h d) -> p h d", d=64)
            nc.vector.tensor_copy(vv3[:, half * 8:half * 8 + 8, 0:64], ps3[:])

        def proj_chunks(m):
            out = [lambda n0=n0: k_chunk(m, n0) for n0 in (0, 512)]
            if k_tail:
                out.append(lambda: k_chunk(m, 1024, k_tail))
            return out

        # ---- attention: units of (head pair j, query half qh) ----------
        # Emission is fused at kt granularity: each kt step of unit u carries
        # the energy matmuls + exp of (u, kt) AND the AV matmuls of unit u-1
        # at kt, so the in-order PE queue always has AV filler between
        # energy steps while the Act engine's exp stream catches up.

        def emit_av_kt(st, kt):
            """AV matmuls at key-tile kt for the unit described by `st`."""
            j, qh, ptiles, avs = st["j"], st["qh"], st["pt"], st["avs"]
            for pair in (0, 1):
                av = avs[pair]
                for qi in (0, 1):
                    for hh in (0, 1):
                        idx = kt * 4 + qi * 2 + hh
                        q0 = hh * 512 + (pair * 2 + qi) * 128
                        nc.tensor.matmul(
                            av[:, (qi * 2 + hh) * 65:(qi * 2 + hh + 1) * 65],
                            ptiles[kt][:, q0:q0 + 128],
                            VVs[kt][:, (2 * j + hh) * 65:(2 * j + hh + 1) * 65],
                            start=(idx == 0), stop=(idx == KTn * 4 - 1),
                            skip_group_check=True)

        def finalize_av(st):
            """Normalize + transpose the finished AV unit `st`."""
            j, qh, avs, oj = st["j"], st["qh"], st["avs"], st["oj"]
            for pair in (0, 1):
                av = avs[pair]
                av3 = av[:].rearrange("p (x c) -> p x c", c=65)
                rc = nrm.tile([128, 4], F32, tag="rc",
                              name=f"rc{j}_{qh}_{pair}")
                nc.vector.reciprocal(
                    rc[:].rearrange("p (a b) -> p a b", b=1), av3[:, :, 64:65])
                for qi in (0, 1):
                    ql = pair * 2 + qi
                    qc = qh * 4 + ql
                    for hh in (0, 1):
                        i = qi * 2 + hh
                        nc.vector.tensor_scalar_mul(
                            oj[:, ql, hh * 64:hh * 64 + 64],
                            av[:, i * 65:i * 65 + 64], rc[:, i:i + 1])
                    nc.sync.dma_start_transpose(
                        OTs[j][:, qc * 128:(qc + 1) * 128], oj[:, ql, :])

        HIPRI = 1 << 20
        for k in range(EC):
            for n0 in (0, 512):
                q_chunk(0, n0, k, 1, k == 0)
        # all remaining Q projections precede Kp(0) so the psF buffer
        # rotation never stalls behind the (late) wk DMA
        for m in range(1, EC):
            q_chunk(m, 0)
            q_chunk(m, 512)
        k_chunk(0, 0)
        k_chunk(0, 512)
        if k_tail:
            k_chunk(0, 1024, k_tail)
        units = [(j, qh) for j in range(EC) for qh in (0, 1)]
        wo_t = [None]
        prev = None
        for j, qh in units:
            ptiles = []
            chunks = (proj_chunks(j + 1)
                      if (qh == 0 and j < EC - 1) else [])
            if (j, qh) == (0, 1):
                # V projection must be emitted before any AV(0,0) matmul:
                # dependency tracking is program-order directional, so a
                # read emitted before its writer would race it.
                for t in range(KTn):
                    v_chunk(t, 0)
                    v_chunk(t, 1)
                inpV_ctx.close()
                wo_pool = ctx.enter_context(tc.tile_pool(name="wop",
                                                         bufs=1))
                wo_t[0] = wo_pool.tile([128, EC, E], BF16, name="wo")
                nc.sync.dma_start(
                    wo_t[0][:],
                    woT[:].rearrange("(c p) e -> p c e", p=128))
            for kt in range(KTn):
                with tc.high_priority(offset=HIPRI):
                    pe = psE.tile([128, 1024], F32, tag="e",
                                  name=f"pe{j}_{qh}_{kt}")
                    for hh in (0, 1):
                        off = hh * 64
                        nc.tensor.matmul(
                            pe[:, hh * 512:hh * 512 + 512],
                            KTs[j][off:off + 64, kt * 128:(kt + 1) * 128],
                            QTs[j][off:off + 64, qh * 512:qh * 512 + 512])
                    pt = pp.tile([128, 1024], BF16, tag=f"P{qh}_{kt}",
                                 name=f"pt{j}_{qh}_{kt}")
                    nc.scalar.activation(
                        pt[:], pe[:], mybir.ActivationFunctionType.Exp,
                        bias=mb_s[:, kt:kt + 1], scale=0.125)
                    ptiles.append(pt)
                    if prev is not None:
                        if kt == 0:
                            prev["avs"] = [
                                psA.tile([128, 260], F32, tag=f"a{pr}",
                                         name=f"av{prev['j']}_{prev['qh']}_{pr}")
                                for pr in (0, 1)]
                        emit_av_kt(prev, kt)
                if kt % 2 == 1 and chunks:
                    chunks.pop(0)()
            while chunks:
                chunks.pop(0)()
            if prev is not None:
                with tc.high_priority(offset=HIPRI):
                    finalize_av(prev)
            prev = dict(j=j, qh=qh, pt=ptiles, avs=None,
                        oj=nrm.tile([128, 4, 128], BF16, tag="oj",
                                    name=f"oj{j}_{qh}"))
        with tc.high_priority(offset=HIPRI):
            prev["avs"] = [psA.tile([128, 260], F32, tag=f"a{pr}",
                                    name=f"av_last_{pr}") for pr in (0, 1)]
            for kt in range(KTn):
                emit_av_kt(prev, kt)
            finalize_av(prev)

        out_pool = ctx.enter_context(tc.tile_pool(name="outp", bufs=2))

        # ---- output projection Y^T = Wo @ O^T + bo' --------------------
        for m in range(EC):
            ps = psE.tile([128, 1024], F32, tag="e", name=f"psy{m}")
            for n0 in (0, 512):
                for k in range(EC):
                    nc.tensor.matmul(
                        ps[:, n0:n0 + 512],
                        wo_t[0][:, k, m * 128:(m + 1) * 128],
                        OTs[k][:, n0:n0 + 512],
                        start=(k == 0), stop=(k == EC - 1))
            yt = out_pool.tile([128, Q], F32, tag="yt", name=f"yt{m}")
            for n0 in (0, 512):
                nc.vector.tensor_scalar_add(
                    yt[:, n0:n0 + 512], ps[:, n0:n0 + 512], bo_s[:, m:m + 1])
                nc.sync.dma_start(yT[m * 128:(m + 1) * 128, n0:n0 + 512],
                                  yt[:, n0:n0 + 512])

    nc.compile()
    return nc


_PROG_CACHE = {}


def _get_program(Kpad):
    if Kpad not in _PROG_CACHE:
        _PROG_CACHE[Kpad] = build_program(Kpad)
    return _PROG_CACHE[Kpad]


def prepare_inputs(query, keys, values, mask, Wq, bq, Wk, bk, Wv, bv, Wo, bo):
    """Host-side sharding/layout prep. Returns (Kpad, in_maps)."""
    f32 = np.float32
    query = np.asarray(query, f32)
    keys = np.asarray(keys, f32)
    values = np.asarray(values, f32)
    mask = np.asarray(mask)

    idxs = [np.nonzero(mask[b] != 0)[0] for b in range(B)]
    nmax = max(len(i) for i in idxs)
    Kpad = max(256, ((max(nmax, 1) + 127) // 128) * 128)
    KTn = Kpad // 128

    kTb = np.zeros((B, E, Kpad), BF16NP)
    vTb = np.zeros((B, E, Kpad), BF16NP)
    mbb = np.full((B, Kpad), -1e9, f32)
    for b in range(B):
        n = len(idxs[b])
        kTb[b, :, :n] = keys[b][idxs[b]].T.astype(BF16NP)
        vTb[b, :, :n] = values[b][idxs[b]].T.astype(BF16NP)
        mbb[b, :n] = 0.0
    mb2 = np.ascontiguousarray(mbb.reshape(B, KTn, 128).transpose(0, 2, 1))

    WqT = np.ascontiguousarray(np.asarray(Wq, f32).T.astype(BF16NP))
    WkT = np.ascontiguousarray(np.asarray(Wk, f32).T.astype(BF16NP))
    WvT = np.ascontiguousarray(np.asarray(Wv, f32).T.astype(BF16NP))
    WoT = np.ascontiguousarray(np.asarray(Wo, f32).T.astype(BF16NP))
    bq2 = np.ascontiguousarray(np.asarray(bq, f32).reshape(EC, 128).T)
    bk2 = np.ascontiguousarray(np.asarray(bk, f32).reshape(EC, 128).T)
    # fold V bias through the output projection: y += (Wo @ bv + bo)
    bo_f = np.asarray(bo, f32) + np.asarray(Wo, f32) @ np.asarray(bv, f32)
    bo2 = np.ascontiguousarray(bo_f.reshape(EC, 128).T)

    in_maps = []
    for c in range(NCORES):
        b, h = c // 2, c % 2
        in_maps.append(dict(
            qT=np.ascontiguousarray(
                query[b, h * Q:(h + 1) * Q, :].T.astype(BF16NP)),
            kT=kTb[b], vT=vTb[b], mb=mb2[b],
            wqT=WqT, wkT=WkT, wvT=WvT, woT=WoT,
            bq2=bq2, bk2=bk2, bo2=bo2,
        ))
    return Kpad, in_maps


def kernel(query, keys, values, mask, Wq, bq, Wk, bk, Wv, bv, Wo, bo):
    Kpad, in_maps = prepare_inputs(query, keys, values, mask,
                                   Wq, bq, Wk, bk, Wv, bv, Wo, bo)
    nc = _get_program(Kpad)
    res = run_bass_kernel_spmd(nc, in_maps, list(range(NCORES)))
    out = np.empty((B, S, E), np.float32)
    for c in range(NCORES):
        b, h = c // 2, c % 2
        out[b, h * Q:(h + 1) * Q, :] = res.results[c]["yT"].T
    return out


# revision 41
# speedup vs baseline: 1.0496x; 1.0007x over previous
"""Multi-head attention on 8 TRN2 NeuronCores (Bass/Tile).

Sharding: core c handles batch b = c//2 and query-half h = c%2 (1024 query
tokens), all 16 heads. K/V projections are per-batch and duplicated across
the two cores sharing a batch; no cross-core communication.

Design notes (v3):
- All matmul operands are bf16 (1 PE cycle/row at any output width, vs
  fp32r's 4x penalty below 256), halving DMA traffic as well. PSUM stays
  fp32.
- Keys are compacted on the host using the 0/1 key mask (masked keys
  dropped, padded to a multiple of 128, Kpad). Pad positions are killed by
  a -1e9 per-partition bias folded into the Exp activation.
- The V-projection bias is folded into the output-projection bias on the
  host (bo' = bo + Wo @ bv), since softmax weights sum to 1.
- Energy is computed transposed ([key, query] tiles). AV is computed as
  out[q-tile, 65] accumulating over key tiles: each head's V tile carries
  an extra ones column, so column 64 of the PSUM output is the softmax
  denominator, a per-partition scalar. Normalization is then a [128,k]
  reciprocal plus per-head tensor_scalar multiplies - no broadcast matmul.
  Normalized O ([q, e] layout) returns to [e, q] tiles for the output
  projection via XBAR DMA transposes on the otherwise-idle SP queue.
- Attention runs in (head-pair, query-half) units, fused at key-tile
  granularity: each kt step carries the energy matmuls + exp of (u, kt)
  and the AV matmuls of unit u-1, all at high scheduler priority, while
  the Q/K/V projections are emitted as ~1.7us single-bank PSUM chunks
  that fill PE gaps whenever the Act engine's exp stream (the secondary
  bottleneck, ~150us) falls behind.
- All Q projections precede Kp(0) so the filler-PSUM rotation never
  stalls behind the late wk DMA; emission order matters beyond
  priorities because tile dependency tracking is program-order
  directional (a read emitted before its writer races it).
"""

import sys

sys.path.insert(0, "/opt/trn_rl_repo")

from contextlib import ExitStack

import ml_dtypes
import numpy as np

import concourse.bass as bass  # noqa: F401
import concourse.tile as tile
from concourse import bacc, mybir
from concourse.bass_utils import run_bass_kernel_spmd

E = 1024          # embed dim
HEADS = 16
HD = 64           # head dim
B = 4
S = 2048
NCORES = 8
Q = (B * S) // NCORES  # query tokens per core
EC = E // 128     # embed chunks of 128
F32 = mybir.dt.float32
BF16 = mybir.dt.bfloat16
BF16NP = ml_dtypes.bfloat16


def build_program(Kpad):
    """Build the per-core Bass program (identical on all 8 cores)."""
    KTn = Kpad // 128
    nc = bacc.Bacc("TRN2", target_bir_lowering=False, debug=False,
                   num_devices=NCORES, dynamic_dma_scratch_size=2048)

    qT = nc.dram_tensor("qT", [E, Q], BF16, kind="ExternalInput").ap()
    kT = nc.dram_tensor("kT", [E, Kpad], BF16, kind="ExternalInput").ap()
    vT = nc.dram_tensor("vT", [E, Kpad], BF16, kind="ExternalInput").ap()
    wqT = nc.dram_tensor("wqT", [E, E], BF16, kind="ExternalInput").ap()
    wkT = nc.dram_tensor("wkT", [E, E], BF16, kind="ExternalInput").ap()
    wvT = nc.dram_tensor("wvT", [E, E], BF16, kind="ExternalInput").ap()
    woT = nc.dram_tensor("woT", [E, E], BF16, kind="ExternalInput").ap()
    bq2 = nc.dram_tensor("bq2", [128, EC], F32, kind="ExternalInput").ap()
    bk2 = nc.dram_tensor("bk2", [128, EC], F32, kind="ExternalInput").ap()
    bo2 = nc.dram_tensor("bo2", [128, EC], F32, kind="ExternalInput").ap()
    mb = nc.dram_tensor("mb", [128, KTn], F32, kind="ExternalInput").ap()
    yT = nc.dram_tensor("yT", [E, Q], F32, kind="ExternalOutput").ap()

    # K-projection free-dim chunks (each must stay inside a 512-float bank)
    k_tail = Kpad - 1024 if Kpad > 1024 else 0

    with tile.TileContext(nc) as tc, ExitStack() as ctx:
        sml = ctx.enter_context(tc.tile_pool(name="sml", bufs=1))
        big = ctx.enter_context(tc.tile_pool(name="big", bufs=1))

        # ---- PSUM pools: psE 2x[128,1024]=4 banks, psA 2x1=2, psT 2x1=2
        psE = ctx.enter_context(tc.tile_pool(name="psE", bufs=2, space="PSUM"))
        psA = ctx.enter_context(tc.tile_pool(name="psA", bufs=1, space="PSUM"))
        psF = ctx.enter_context(tc.tile_pool(name="psF", bufs=2, space="PSUM"))

        inp = ctx.enter_context(tc.tile_pool(name="inp", bufs=1))
        pp = ctx.enter_context(tc.tile_pool(name="pp", bufs=1))
        nrm = ctx.enter_context(tc.tile_pool(name="nrm", bufs=4))
        # vt/wv free mid-attention; inpV sits atop the pool stack so it can
        inpV_ctx = ExitStack()
        inpV = inpV_ctx.enter_context(tc.tile_pool(name="inpV", bufs=1))

        # ---- big input DMAs (SP queue), in consumption order -----------
        # qT/wq are DMA'd per 128-row chunk so the first matmuls start
        # early; the small constants go out after the first two chunk
        # pairs (they are only needed once the first PSUM drains).
        qt_t, wq_t = [], []
        qv = qT[:].rearrange("(c p) q -> p c q", p=128)
        wv = wqT[:].rearrange("(c p) e -> p c e", p=128)
        bq_s = sml.tile([128, EC], F32, name="bq_s")
        bk_s = sml.tile([128, EC], F32, name="bk_s")
        bo_s = sml.tile([128, EC], F32, name="bo_s")
        mb_s = sml.tile([128, KTn], F32, name="mb_s")
        for h in (0, 1):
            t = inp.tile([128, 4, Q], BF16, name=f"qt{h}")
            w = inp.tile([128, 4, E], BF16, name=f"wq{h}")
            for kk in range(4):
                nc.sync.dma_start(t[:, kk, :], qv[:, 4 * h + kk, :])
                nc.sync.dma_start(w[:, kk, :], wv[:, 4 * h + kk, :])
                if h == 0 and kk == 1:
                    nc.sync.dma_start(bq_s[:], bq2[:])
                    nc.sync.dma_start(bk_s[:], bk2[:])
                    nc.sync.dma_start(bo_s[:], bo2[:])
                    nc.sync.dma_start(mb_s[:], mb[:])
            qt_t.append(t)
            wq_t.append(w)
        kt_t = inp.tile([128, EC, Kpad], BF16, name="kt")
        nc.sync.dma_start(kt_t[:], kT[:].rearrange("(c p) k -> p c k", p=128))
        wk_t = inp.tile([128, EC, E], BF16, name="wk")
        nc.sync.dma_start(wk_t[:], wkT[:].rearrange("(c p) e -> p c e", p=128))
        vt_t = inpV.tile([128, EC, Kpad], BF16, name="vt")
        nc.sync.dma_start(vt_t[:], vT[:].rearrange("(c p) k -> p c k", p=128))
        wv_t = inpV.tile([128, EC, E], BF16, name="wv")
        nc.sync.dma_start(wv_t[:], wvT[:].rearrange("(c p) e -> p c e", p=128))

        # ---- persistent SBUF tensors -----------------------------------
        QTs = [big.tile([128, Q], BF16, name=f"QT{m}") for m in range(EC)]
        KTs = [big.tile([128, Kpad], BF16, name=f"KT{m}") for m in range(EC)]
        VVs = [big.tile([128, HEADS * 65], BF16, name=f"VV{t}")
               for t in range(KTn)]
        OTs = [big.tile([128, Q], BF16, name=f"OT{m}") for m in range(EC)]
        # ones columns of the V tiles, written once before anything reads VV
        for t in range(KTn):
            vv3 = VVs[t][:].rearrange("p (h e) -> p h e", e=65)
            nc.vector.memset(vv3[:, :, 64:65], 1.0)

        # ---- projection emitters ---------------------------------------
        # All projections run as 8-matmul chunks into a single-bank psF
        # tile that is consumed (DVE) right away, so low-priority filler
        # never blocks the high-priority energy PSUM rotation.
        def q_chunk(m, n0, k0=0, nk=EC, first=True):
            ps = psF.tile([128, 512], F32, tag="f", name=f"psq{m}_{n0}_{k0}")
            for kk in range(k0, k0 + nk):
                nc.tensor.matmul(
                    ps[:], wq_t[kk // 4][:, kk % 4, m * 128:(m + 1) * 128],
                    qt_t[kk // 4][:, kk % 4, n0:n0 + 512],
                    start=(kk == k0), stop=(kk == k0 + nk - 1))
            if first:
                nc.vector.tensor_scalar_add(
                    QTs[m][:, n0:n0 + 512], ps[:], bq_s[:, m:m + 1])
            else:
                with nc.allow_low_precision(reason="bf16 proj accum"):
                    nc.vector.tensor_add(
                        QTs[m][:, n0:n0 + 512], QTs[m][:, n0:n0 + 512], ps[:])

        def k_chunk(m, n0, nn=512):
            ps = psF.tile([128, 512], F32, tag="f", name=f"psk{m}_{n0}")
            for kk in range(EC):
                nc.tensor.matmul(
                    ps[:, 0:nn], wk_t[:, kk, m * 128:(m + 1) * 128],
                    kt_t[:, kk, n0:n0 + nn],
                    start=(kk == 0), stop=(kk == EC - 1))
            nc.vector.tensor_scalar_add(
                KTs[m][:, n0:n0 + nn], ps[:, 0:nn], bk_s[:, m:m + 1])

        def v_chunk(t, half):
            """V-proj chunk: heads half*8..half*8+8 of key tile t."""
            n0 = half * 512
            ps = psF.tile([128, 512], F32, tag="f", name=f"psv{t}_{half}")
            for kk in range(EC):
                nc.tensor.matmul(
                    ps[:], vt_t[:, kk, t * 128:(t + 1) * 128],
                    wv_t[:, kk, n0:n0 + 512],
                    start=(kk == 0), stop=(kk == EC - 1))
            vv3 = VVs[t][:].rearrange("p (h e) -> p h e", e=65)
            ps3 = ps[:].rearrange("p (h d) -> p h d", d=64)
            nc.vector.tensor_copy(vv3[:, half * 8:half * 8 + 8, 0:64], ps3[:])

        def proj_chunks(m):
            out = [lambda n0=n0: k_chunk(m, n0) for n0 in (0, 512)]
            if k_tail:
                out.append(lambda: k_chunk(m, 1024, k_tail))
            return out

        # ---- attention: units of (head pair j, query half qh) ----------
        # Emission is fused at kt granularity: each kt step of unit u carries
        # the energy matmuls + exp of (u, kt) AND the AV matmuls of unit u-1
        # at kt, so the in-order PE queue always has AV filler between
        # energy steps while the Act engine's exp stream catches up.

        def emit_av_kt(st, kt):
            """AV matmuls at key-tile kt for the unit described by `st`."""
            j, qh, ptiles, avs = st["j"], st["qh"], st["pt"], st["avs"]
            for pair in (0, 1):
                av = avs[pair]
                for qi in (0, 1):
                    for hh in (0, 1):
                        idx = kt * 4 + qi * 2 + hh
                        q0 = hh * 512 + (pair * 2 + qi) * 128
                        nc.tensor.matmul(
                            av[:, (qi * 2 + hh) * 65:(qi * 2 + hh + 1) * 65],
                            ptiles[kt][:, q0:q0 + 128],
                            VVs[kt][:, (2 * j + hh) * 65:(2 * j + hh + 1) * 65],
                            start=(idx == 0), stop=(idx == KTn * 4 - 1),
                            skip_group_check=True)

        def finalize_av(st):
            """Normalize + transpose the finished AV unit `st`."""
            j, qh, avs, oj = st["j"], st["qh"], st["avs"], st["oj"]
            for pair in (0, 1):
                av = avs[pair]
                av3 = av[:].rearrange("p (x c) -> p x c", c=65)
                rc = nrm.tile([128, 4], F32, tag="rc",
                              name=f"rc{j}_{qh}_{pair}")
                nc.vector.reciprocal(
                    rc[:].rearrange("p (a b) -> p a b", b=1), av3[:, :, 64:65])
                for qi in (0, 1):
                    ql = pair * 2 + qi
                    qc = qh * 4 + ql
                    for hh in (0, 1):
                        i = qi * 2 + hh
                        nc.vector.tensor_scalar_mul(
                            oj[:, ql, hh * 64:hh * 64 + 64],
                            av[:, i * 65:i * 65 + 64], rc[:, i:i + 1])
                    nc.sync.dma_start_transpose(
                        OTs[j][:, qc * 128:(qc + 1) * 128], oj[:, ql, :])

        HIPRI = 1 << 20
        for k in range(EC):
            for n0 in (0, 512):
                q_chunk(0, n0, k, 1, k == 0)
        # all remaining Q projections precede Kp(0) so the psF buffer
        # rotation never stalls behind the (late) wk DMA
        for m in range(1, EC):
            q_chunk(m, 0)
            q_chunk(m, 512)
        # Kp(0) at high priority: its matmuls and DVE bias-add preempt the
        # queued Q work the moment wk lands, starting the Act-critical
        # exp chain ~11us earlier.
        with tc.high_priority(offset=HIPRI):
            k_chunk(0, 0)
            k_chunk(0, 512)
            if k_tail:
                k_chunk(0, 1024, k_tail)
        units = [(j, qh) for j in range(EC) for qh in (0, 1)]
        wo_t = [None]
        prev = None
        for j, qh in units:
            ptiles = []
            chunks = (proj_chunks(j + 1)
                      if (qh == 0 and j < EC - 1) else [])
            if (j, qh) == (0, 1):
                # V projection must be emitted before any AV(0,0) matmul:
                # dependency tracking is program-order directional, so a
                # read emitted before its writer would race it.
                for t in range(KTn):
                    v_chunk(t, 0)
                    v_chunk(t, 1)
                inpV_ctx.close()
                wo_pool = ctx.enter_context(tc.tile_pool(name="wop",
                                                         bufs=1))
                wo_t[0] = wo_pool.tile([128, EC, E], BF16, name="wo")
                nc.sync.dma_start(
                    wo_t[0][:],
                    woT[:].rearrange("(c p) e -> p c e", p=128))
            for kt in range(KTn):
                with tc.high_priority(offset=HIPRI):
                    pe = psE.tile([128, 1024], F32, tag="e",
                                  name=f"pe{j}_{qh}_{kt}")
                    for hh in (0, 1):
                        off = hh * 64
                        nc.tensor.matmul(
                            pe[:, hh * 512:hh * 512 + 512],
                            KTs[j][off:off + 64, kt * 128:(kt + 1) * 128],
                            QTs[j][off:off + 64, qh * 512:qh * 512 + 512])
                    pt = pp.tile([128, 1024], BF16, tag=f"P{qh}_{kt}",
                                 name=f"pt{j}_{qh}_{kt}")
                    nc.scalar.activation(
                        pt[:], pe[:], mybir.ActivationFunctionType.Exp,
                        bias=mb_s[:, kt:kt + 1], scale=0.125)
                    ptiles.append(pt)
                    if prev is not None:
                        if kt == 0:
                            prev["avs"] = [
                                psA.tile([128, 260], F32, tag=f"a{pr}",
                                         name=f"av{prev['j']}_{prev['qh']}_{pr}")
                                for pr in (0, 1)]
                        emit_av_kt(prev, kt)
                if kt % 2 == 1 and chunks:
                    chunks.pop(0)()
            while chunks:
                chunks.pop(0)()
            if prev is not None:
                with tc.high_priority(offset=HIPRI):
                    finalize_av(prev)
            prev = dict(j=j, qh=qh, pt=ptiles, avs=None,
                        oj=nrm.tile([128, 4, 128], BF16, tag="oj",
                                    name=f"oj{j}_{qh}"))
        with tc.high_priority(offset=HIPRI):
            prev["avs"] = [psA.tile([128, 260], F32, tag=f"a{pr}",
                                    name=f"av_last_{pr}") for pr in (0, 1)]
            for kt in range(KTn):
                emit_av_kt(prev, kt)
            finalize_av(prev)

        out_pool = ctx.enter_context(tc.tile_pool(name="outp", bufs=2))

        # ---- output projection Y^T = Wo @ O^T + bo' --------------------
        for m in range(EC):
            ps = psE.tile([128, 1024], F32, tag="e", name=f"psy{m}")
            for n0 in (0, 512):
                for k in range(EC):
                    nc.tensor.matmul(
                        ps[:, n0:n0 + 512],
                        wo_t[0][:, k, m * 128:(m + 1) * 128],
                        OTs[k][:, n0:n0 + 512],
                        start=(k == 0), stop=(k == EC - 1))
            yt = out_pool.tile([128, Q], F32, tag="yt", name=f"yt{m}")
            for n0 in (0, 512):
                nc.vector.tensor_scalar_add(
                    yt[:, n0:n0 + 512], ps[:, n0:n0 + 512], bo_s[:, m:m + 1])
                nc.sync.dma_start(yT[m * 128:(m + 1) * 128, n0:n0 + 512],
                                  yt[:, n0:n0 + 512])

    nc.compile()
    return nc


_PROG_CACHE = {}


def _get_program(Kpad):
    if Kpad not in _PROG_CACHE:
        _PROG_CACHE[Kpad] = build_program(Kpad)
    return _PROG_CACHE[Kpad]


def prepare_inputs(query, keys, values, mask, Wq, bq, Wk, bk, Wv, bv, Wo, bo):
    """Host-side sharding/layout prep. Returns (Kpad, in_maps)."""
    f32 = np.float32
    query = np.asarray(query, f32)
    keys = np.asarray(keys, f32)
    values = np.asarray(values, f32)
    mask = np.asarray(mask)

    idxs = [np.nonzero(mask[b] != 0)[0] for b in range(B)]
    nmax = max(len(i) for i in idxs)
    Kpad = max(256, ((max(nmax, 1) + 127) // 128) * 128)
    KTn = Kpad // 128

    kTb = np.zeros((B, E, Kpad), BF16NP)
    vTb = np.zeros((B, E, Kpad), BF16NP)
    mbb = np.full((B, Kpad), -1e9, f32)
    for b in range(B):
        n = len(idxs[b])
        kTb[b, :, :n] = keys[b][idxs[b]].T.astype(BF16NP)
        vTb[b, :, :n] = values[b][idxs[b]].T.astype(BF16NP)
        mbb[b, :n] = 0.0
    mb2 = np.ascontiguousarray(mbb.reshape(B, KTn, 128).transpose(0, 2, 1))

    WqT = np.ascontiguousarray(np.asarray(Wq, f32).T.astype(BF16NP))
    WkT = np.ascontiguousarray(np.asarray(Wk, f32).T.astype(BF16NP))
    WvT = np.ascontiguousarray(np.asarray(Wv, f32).T.astype(BF16NP))
    WoT = np.ascontiguousarray(np.asarray(Wo, f32).T.astype(BF16NP))
    bq2 = np.ascontiguousarray(np.asarray(bq, f32).reshape(EC, 128).T)
    bk2 = np.ascontiguousarray(np.asarray(bk, f32).reshape(EC, 128).T)
    # fold V bias through the output projection: y += (Wo @ bv + bo)
    bo_f = np.asarray(bo, f32) + np.asarray(Wo, f32) @ np.asarray(bv, f32)
    bo2 = np.ascontiguousarray(bo_f.reshape(EC, 128).T)

    in_maps = []
    for c in range(NCORES):
        b, h = c // 2, c % 2
        in_maps.append(dict(
            qT=np.ascontiguousarray(
                query[b, h * Q:(h + 1) * Q, :].T.astype(BF16NP)),
            kT=kTb[b], vT=vTb[b], mb=mb2[b],
            wqT=WqT, wkT=WkT, wvT=WvT, woT=WoT,
            bq2=bq2, bk2=bk2, bo2=bo2,
        ))
    return Kpad, in_maps


def kernel(query, keys, values, mask, Wq, bq, Wk, bk, Wv, bv, Wo, bo):
    Kpad, in_maps = prepare_inputs(query, keys, values, mask,
                                   Wq, bq, Wk, bk, Wv, bv, Wo, bo)
    nc = _get_program(Kpad)
    res = run_bass_kernel_spmd(nc, in_maps, list(range(NCORES)))
    out = np.empty((B, S, E), np.float32)
    for c in range(NCORES):
        b, h = c // 2, c % 2
        out[b, h * Q:(h + 1) * Q, :] = res.results[c]["yT"].T
    return out
